# revision 11
# baseline (speedup 1.0000x reference)
"""GCN layer (DGL GraphConv norm='both' + relu + dropout) on 8 trn2 NeuronCores.

Strategy: partition the 50000 nodes into 8 contiguous ranges of 6250 (one per
core). Every edge is owned by the core owning its dst node. Host-side prep is
index manipulation only (sort edges by dst, build padded per-block tile
arrays, degree bincounts -> per-edge norm weights); all feature math happens
on device:

  stage 1: batched dma_gather of feat[src] rows (feat is split into two
           <32768-row halves because gather indices are int16), weighted
           one-hot built on DVE in a single dual-scalar tensor_scalar op,
           PE matmuls accumulate the *transposed* aggregate
           aggT[fi, dst_local] in PSUM.
  stage 2: project with W (PE), relu(y+b) on ACT, dropout-mask multiply on
           DVE, PE transpose back to [node, feat], DMA out.

The dropout mask is a compile-time constant (jax threefry key 42, shape-only).
"""

import math
import os
import sys

import numpy as np

for _p in ("/opt/trn_rl_repo",):
    if _p not in sys.path and os.path.isdir(_p):
        sys.path.insert(0, _p)

N_NODES = 50000
N_EDGES = 400000
D = 256
DROPOUT_P = 0.1
M = 8  # cores
NC_NODES = N_NODES // M  # 6250
P = 128
NB = math.ceil(NC_NODES / P)  # 49 blocks (48 full + 106)
NPAD = NB * P  # 6272 padded nodes per core
HALF = N_NODES // 2  # feat table split for int16 gather indices
GRP = 4  # blocks per gather-call group

_mask_cache = {}


def _dropout_multiplier():
    """Exact replica of the reference's keep mask -> multiplier, f32."""
    if "m" not in _mask_cache:
        import jax

        cpu = jax.devices("cpu")[0]
        with jax.default_device(cpu):
            keep = np.asarray(
                jax.random.bernoulli(
                    jax.random.key(42), 1.0 - DROPOUT_P, (N_NODES, D)
                )
            )
        mult = np.where(
            keep, np.float32(1.0) / np.float32(1.0 - DROPOUT_P), np.float32(0.0)
        ).astype(np.float32)
        _mask_cache["m"] = mult
    return _mask_cache["m"]


def _groups():
    """Group block ids into gather groups of GRP blocks."""
    out = []
    b = 0
    while b < NB:
        out.append(list(range(b, min(b + GRP, NB))))
        b += GRP
    return out


def _wrap16(flat):
    """int16 gather index layout: position n -> [n%16, n//16], tiled to 128
    partitions (Q7 reads its copy per 16-partition group)."""
    C = len(flat) // 16
    a = np.zeros((16, C), dtype=np.int16)
    for n in range(len(flat)):
        a[n % 16, n // 16] = flat[n]
    return np.tile(a, (8, 1))


def _host_prep(src, dst):
    """Sort/shard edges, build per-core gather-index + one-hot arrays.

    Global tile-column order (matches G buffer columns and dw columns):
      for each group: [blk0 lo-tiles.. blkG-1 lo-tiles, blk0 hi-tiles, ...]
    Returns (K_LO, K_HI, per_core list of dicts).
    """
    src = np.asarray(src).astype(np.int64)
    dst = np.asarray(dst).astype(np.int64)

    deg_out = np.bincount(src, minlength=N_NODES).astype(np.float64)
    deg_in = np.bincount(dst, minlength=N_NODES).astype(np.float64)
    w_out = 1.0 / np.sqrt(np.maximum(deg_out, 1.0))
    w_in = 1.0 / np.sqrt(np.maximum(deg_in, 1.0))
    w_edge_all = (w_out[src] * w_in[dst]).astype(np.float32)

    order = np.argsort(dst, kind="stable")
    dst_s = dst[order]
    src_s = src[order]
    w_s = w_edge_all[order]

    # per (core, block, half): edge sublists
    sub = {}
    kmax = [1, 1]
    for c in range(M):
        base = c * NC_NODES
        for b in range(NB):
            lo = base + b * P
            hi = min(base + (b + 1) * P, (c + 1) * NC_NODES)
            s = np.searchsorted(dst_s, lo, side="left")
            e = np.searchsorted(dst_s, hi, side="left")
            sl_src = src_s[s:e]
            sl_dst = dst_s[s:e]
            sl_w = w_s[s:e]
            is_lo = sl_src < HALF
            for h in (0, 1):
                m_ = is_lo if h == 0 else ~is_lo
                sub[(c, b, h)] = (
                    sl_src[m_] - h * HALF,
                    (sl_dst[m_] - lo).astype(np.float32),
                    sl_w[m_],
                )
                kmax[h] = max(kmax[h], math.ceil(m_.sum() / P))
    K_LO, K_HI = kmax
    KT = K_LO + K_HI
    groups = _groups()

    per_core = []
    for c in range(M):
        dstloc = np.full((P, NB * KT), -1.0, dtype=np.float32)
        wedge = np.zeros((P, NB * KT), dtype=np.float32)
        idx_cols = []
        gcol = 0
        for blocks in groups:
            for h, kh in ((0, K_LO), (1, K_HI)):
                flat = np.zeros(len(blocks) * kh * P, dtype=np.int64)
                for gi, b in enumerate(blocks):
                    e_src, e_dl, e_w = sub[(c, b, h)]
                    n = len(e_src)
                    o = gi * kh * P
                    flat[o : o + n] = e_src
                    # dw columns follow the same global column order
                    col0 = gcol + gi * kh
                    for t in range(kh):
                        lo_i = t * P
                        hi_i = min((t + 1) * P, n)
                        if lo_i < n:
                            cnt = hi_i - lo_i
                            dstloc[:cnt, col0 + t] = e_dl[lo_i:hi_i]
                            wedge[:cnt, col0 + t] = e_w[lo_i:hi_i]
                idx_cols.append(_wrap16(flat))
                gcol += len(blocks) * kh
        per_core.append(
            dict(
                idx16=np.ascontiguousarray(np.concatenate(idx_cols, axis=1)),
                dstloc=np.ascontiguousarray(dstloc),
                wedge=np.ascontiguousarray(wedge),
            )
        )
    return K_LO, K_HI, per_core


def _build_program(K_LO, K_HI):
    import concourse.bacc as bacc
    import concourse.mybir as mybir
    from concourse.masks import make_identity
    from concourse.tile import TileContext

    f32 = mybir.dt.float32
    i16 = mybir.dt.int16
    KT = K_LO + K_HI
    NBK = NB * KT
    groups = _groups()
    IDXC = NBK * P // 16  # idx16 total columns

    nc = bacc.Bacc("TRN2")
    featLoD = nc.dram_tensor("featlo", [HALF, D], f32, kind="ExternalInput")
    featHiD = nc.dram_tensor("feathi", [HALF, D], f32, kind="ExternalInput")
    wD = nc.dram_tensor("wmat", [P, 2 * D], f32, kind="ExternalInput")
    bD = nc.dram_tensor("bias2", [P, 2], f32, kind="ExternalInput")
    idxD = nc.dram_tensor("idx16", [P, IDXC], i16, kind="ExternalInput")
    dwD = nc.dram_tensor("dw", [P, 2 * NBK], f32, kind="ExternalInput")
    maskD = nc.dram_tensor("maskt", [D, NPAD], f32, kind="ExternalInput")
    outD = nc.dram_tensor("out", [NPAD, D], f32, kind="ExternalOutput")

    # stage-2 d-chunks
    chunks = []
    d0 = 0
    while d0 < NPAD:
        dn = min(512, NPAD - d0)
        chunks.append((d0, dn))
        d0 += dn

    with TileContext(nc) as tc:
        with (
            tc.tile_pool(name="const", bufs=1) as const,
            tc.tile_pool(name="gpool", bufs=2) as gpool,
            tc.tile_pool(name="ppool", bufs=4) as ppool,
            tc.tile_pool(name="psagg", bufs=2, space="PSUM") as psagg,
            tc.tile_pool(name="ps2", bufs=2, space="PSUM") as ps2p,
            tc.tile_pool(name="pst", bufs=2, space="PSUM") as pstp,
            tc.tile_pool(name="ypool", bufs=4) as ypool,
            tc.tile_pool(name="mpool", bufs=3) as mpool,
            tc.tile_pool(name="opool", bufs=3) as opool,
        ):
            # ---- constants / persistent tiles ----
            w_sb = const.tile([P, 2 * D], f32)  # [fi_in_half, fi_half*256+fo]
            nc.sync.dma_start(out=w_sb[:], in_=wD[:])
            b_sb = const.tile([P, 2], f32)
            nc.sync.dma_start(out=b_sb[:], in_=bD[:])
            iota_sb = const.tile([P, P], f32)
            nc.gpsimd.iota(
                iota_sb[:],
                pattern=[[1, P]],
                base=0,
                channel_multiplier=0,
                allow_small_or_imprecise_dtypes=True,
            )
            ident = const.tile([P, P], f32)
            make_identity(nc, ident[:])
            idx_sb = const.tile([P, IDXC], i16)
            nc.sync.dma_start(out=idx_sb[:], in_=idxD[:])
            dw_sb = const.tile([P, 2 * NBK], f32)  # [dstloc | wedge]
            nc.sync.dma_start(out=dw_sb[:], in_=dwD[:])
            # transposed aggregate: [fi(128) x (fi_half, dst_local 6272)]
            aggT = const.tile([P, 2 * NPAD], f32)

            # ---- stage 1: gather + weighted one-hot + matmul-accumulate ----
            gcol = 0  # global tile column
            icol = 0  # idx16 column offset (int16 cols)
            for blocks in groups:
                ng = len(blocks)
                ncols = ng * KT
                g = gpool.tile([P, ncols * D], f32, tag="g")
                # two gather calls: lo half then hi half
                for h, kh, tbl in ((0, K_LO, featLoD), (1, K_HI, featHiD)):
                    ni = ng * kh * P
                    cpos = 0 if h == 0 else ng * K_LO
                    nc.gpsimd.dma_gather(
                        out_ap=g[:, cpos * D : (cpos + ng * kh) * D].rearrange(
                            "p (k d) -> p k d", k=ng * kh
                        ),
                        in_ap=tbl[:],
                        idxs_ap=idx_sb[:, icol : icol + ni // 16],
                        num_idxs=ni,
                        num_idxs_reg=ni,
                        elem_size=D,
                        single_packet=False,
                    )
                    icol += ni // 16
                for gi, b in enumerate(blocks):
                    pt0 = psagg.tile([P, P], f32, tag="pt0")
                    pt1 = psagg.tile([P, P], f32, tag="pt1")
                    tiles = [  # (G column, dw column) for this block
                        (gi * K_LO + t, gcol + gi * K_LO + t)
                        for t in range(K_LO)
                    ] + [
                        (ng * K_LO + gi * K_HI + t, gcol + ng * K_LO + gi * K_HI + t)
                        for t in range(K_HI)
                    ]
                    for ti, (gc_, dc) in enumerate(tiles):
                        ph = ppool.tile([P, P], f32, tag="ph")
                        # ph[e, s] = (iota[e,s] == dstloc[e]) * wedge[e]
                        nc.vector.tensor_scalar(
                            out=ph[:],
                            in0=iota_sb[:],
                            scalar1=dw_sb[:, dc : dc + 1],
                            scalar2=dw_sb[:, NBK + dc : NBK + dc + 1],
                            op0=mybir.AluOpType.is_equal,
                            op1=mybir.AluOpType.mult,
                        )
                        nc.tensor.matmul(
                            pt0[:],
                            lhsT=g[:, gc_ * D : gc_ * D + P],
                            rhs=ph[:],
                            start=(ti == 0),
                            stop=(ti == KT - 1),
                        )
                        nc.tensor.matmul(
                            pt1[:],
                            lhsT=g[:, gc_ * D + P : (gc_ + 1) * D],
                            rhs=ph[:],
                            start=(ti == 0),
                            stop=(ti == KT - 1),
                        )
                    nc.vector.tensor_copy(aggT[:, b * P : (b + 1) * P], pt0[:])
                    nc.vector.tensor_copy(
                        aggT[:, NPAD + b * P : NPAD + (b + 1) * P], pt1[:]
                    )
                gcol += ncols

            # ---- stage 2: project, relu+bias, mask, transpose, store ----
            for d0, dn in chunks:
                yts = []
                for h in (0, 1):  # fo half
                    ps2 = ps2p.tile([P, 512], f32, tag="ps2")
                    nc.tensor.matmul(
                        ps2[:, :dn],
                        lhsT=w_sb[:, h * P : (h + 1) * P],
                        rhs=aggT[:, d0 : d0 + dn],
                        start=True,
                        stop=False,
                    )
                    nc.tensor.matmul(
                        ps2[:, :dn],
                        lhsT=w_sb[:, D + h * P : D + (h + 1) * P],
                        rhs=aggT[:, NPAD + d0 : NPAD + d0 + dn],
                        start=False,
                        stop=True,
                    )
                    yt = ypool.tile([P, 512], f32, tag="yt")
                    nc.scalar.activation(
                        yt[:, :dn],
                        ps2[:, :dn],
                        mybir.ActivationFunctionType.Relu,
                        bias=b_sb[:, h : h + 1],
                    )
                    mk = mpool.tile([P, 512], f32, tag="mk")
                    nc.sync.dma_start(
                        out=mk[:, :dn], in_=maskD[h * P : (h + 1) * P, d0 : d0 + dn]
                    )
                    nc.vector.tensor_mul(yt[:, :dn], yt[:, :dn], mk[:, :dn])
                    yts.append(yt)
                for jj in range(dn // P):
                    ot = opool.tile([P, D], f32, tag="ot")
                    for h in (0, 1):
                        ptr = pstp.tile([P, P], f32, tag="ptr")
                        nc.tensor.transpose(
                            ptr[:], yts[h][:, jj * P : (jj + 1) * P], ident[:]
                        )
                        nc.vector.tensor_copy(ot[:, h * P : (h + 1) * P], ptr[:])
                    nc.sync.dma_start(
                        out=outD[d0 + jj * P : d0 + (jj + 1) * P, :], in_=ot[:]
                    )
    nc.finalize()  # Bacc: runs compile() (register alloc, wait splitting)
    return nc


def kernel(feat, W, b, src, dst):
    feat = np.ascontiguousarray(np.asarray(feat, dtype=np.float32))
    W = np.asarray(W, dtype=np.float32)
    b = np.asarray(b, dtype=np.float32)

    K_LO, K_HI, per_core = _host_prep(src, dst)
    maskmul = _dropout_multiplier()
    maskT_all = np.ascontiguousarray(maskmul.T)  # [256, 50000]

    # W[fi, fo] -> [fi_in_half(128), fi_half*256 + fo]
    wmat = np.ascontiguousarray(
        W.reshape(2, P, D).transpose(1, 0, 2).reshape(P, 2 * D)
    )
    bias2 = np.ascontiguousarray(np.stack([b[:P], b[P:]], axis=1))
    featlo = np.ascontiguousarray(feat[:HALF])
    feathi = np.ascontiguousarray(feat[HALF:])

    in_maps = []
    for c in range(M):
        maskt = np.zeros((D, NPAD), dtype=np.float32)
        maskt[:, :NC_NODES] = maskT_all[:, c * NC_NODES : (c + 1) * NC_NODES]
        dw = np.concatenate(
            [per_core[c]["dstloc"], per_core[c]["wedge"]], axis=1
        )
        in_maps.append(
            dict(
                featlo=featlo,
                feathi=feathi,
                wmat=wmat,
                bias2=bias2,
                idx16=per_core[c]["idx16"],
                dw=np.ascontiguousarray(dw),
                maskt=maskt,
            )
        )

    nc = _build_program(K_LO, K_HI)

    from concourse.bass_utils import run_bass_kernel_spmd

    trace = bool(int(os.environ.get("KERNEL_TRACE", "0")))
    tmpdir = os.environ.get("KERNEL_TMPDIR") or None
    res = run_bass_kernel_spmd(
        nc, in_maps, core_ids=list(range(M)), trace=trace, tmpdir=tmpdir
    )
    if trace:
        kernel.last_exec_time_ns = res.exec_time_ns
        kernel.last_profile = res.profile_json

    out = np.empty((N_NODES, D), dtype=np.float32)
    for c in range(M):
        out[c * NC_NODES : (c + 1) * NC_NODES] = res.results[c]["out"][:NC_NODES]
    return out


# revision 16
# speedup vs baseline: 1.2154x; 1.2154x over previous
"""GCN layer (DGL GraphConv norm='both' + relu + dropout) on 8 trn2 NeuronCores.

Strategy: partition the 50000 nodes into 8 contiguous ranges of 6250 (one per
core). Every edge is owned by the core owning its dst node. Host-side prep is
index manipulation only (sort edges by dst, build padded per-block tile
arrays, degree bincounts -> per-edge norm weights); all feature math happens
on device:

  stage 1: batched dma_gather of feat[src] rows (feat is split into two
           <32768-row halves because gather indices are int16), weighted
           one-hot built on DVE in a single dual-scalar tensor_scalar op,
           PE matmuls accumulate the *transposed* aggregate
           aggT[fi, dst_local] in PSUM.
  stage 2: project with W (PE), relu(y+b) on ACT, dropout-mask multiply on
           DVE, PE transpose back to [node, feat], DMA out.

The dropout mask is a compile-time constant (jax threefry key 42, shape-only).
"""

import math
import os
import sys

import numpy as np

for _p in ("/opt/trn_rl_repo",):
    if _p not in sys.path and os.path.isdir(_p):
        sys.path.insert(0, _p)

N_NODES = 50000
N_EDGES = 400000
D = 256
DROPOUT_P = 0.1
M = 8  # cores
NC_NODES = N_NODES // M  # 6250
P = 128
NB = math.ceil(NC_NODES / P)  # 49 blocks (48 full + 106)
NPAD = NB * P  # 6272 padded nodes per core
HALF = N_NODES // 2  # feat table split for int16 gather indices
GRP = 4  # blocks per gather-call group

_mask_cache = {}


def _dropout_multiplier():
    """Exact replica of the reference's keep mask -> multiplier, f32."""
    if "m" not in _mask_cache:
        import jax

        cpu = jax.devices("cpu")[0]
        with jax.default_device(cpu):
            keep = np.asarray(
                jax.random.bernoulli(
                    jax.random.key(42), 1.0 - DROPOUT_P, (N_NODES, D)
                )
            )
        mult = np.where(
            keep, np.float32(1.0) / np.float32(1.0 - DROPOUT_P), np.float32(0.0)
        ).astype(np.float32)
        _mask_cache["m"] = mult
    return _mask_cache["m"]


def _groups():
    """Group block ids into gather groups of GRP blocks."""
    out = []
    b = 0
    while b < NB:
        out.append(list(range(b, min(b + GRP, NB))))
        b += GRP
    return out


def _wrap16(flat):
    """int16 gather index layout: position n -> [n%16, n//16], tiled to 128
    partitions (Q7 reads its copy per 16-partition group)."""
    C = len(flat) // 16
    a = np.zeros((16, C), dtype=np.int16)
    for n in range(len(flat)):
        a[n % 16, n // 16] = flat[n]
    return np.tile(a, (8, 1))


def _host_prep(src, dst):
    """Sort/shard edges, build per-core gather-index + one-hot arrays.

    Global tile-column order (matches G buffer columns and dw columns):
      for each group: [blk0 lo-tiles.. blkG-1 lo-tiles, blk0 hi-tiles, ...]
    Returns (K_LO, K_HI, per_core list of dicts).
    """
    src = np.asarray(src).astype(np.int64)
    dst = np.asarray(dst).astype(np.int64)

    deg_out = np.bincount(src, minlength=N_NODES).astype(np.float64)
    deg_in = np.bincount(dst, minlength=N_NODES).astype(np.float64)
    w_out = 1.0 / np.sqrt(np.maximum(deg_out, 1.0))
    w_in = 1.0 / np.sqrt(np.maximum(deg_in, 1.0))
    w_edge_all = (w_out[src] * w_in[dst]).astype(np.float32)

    order = np.argsort(dst, kind="stable")
    dst_s = dst[order]
    src_s = src[order]
    w_s = w_edge_all[order]

    # per (core, block, half): edge sublists
    sub = {}
    kmax = [1, 1]
    for c in range(M):
        base = c * NC_NODES
        for b in range(NB):
            lo = base + b * P
            hi = min(base + (b + 1) * P, (c + 1) * NC_NODES)
            s = np.searchsorted(dst_s, lo, side="left")
            e = np.searchsorted(dst_s, hi, side="left")
            sl_src = src_s[s:e]
            sl_dst = dst_s[s:e]
            sl_w = w_s[s:e]
            is_lo = sl_src < HALF
            for h in (0, 1):
                m_ = is_lo if h == 0 else ~is_lo
                sub[(c, b, h)] = (
                    sl_src[m_] - h * HALF,
                    (sl_dst[m_] - lo).astype(np.float32),
                    sl_w[m_],
                )
                kmax[h] = max(kmax[h], math.ceil(m_.sum() / P))
    K_LO, K_HI = kmax
    KT = K_LO + K_HI
    groups = _groups()

    per_core = []
    for c in range(M):
        dstloc = np.full((P, NB * KT), -1.0, dtype=np.float32)
        wedge = np.zeros((P, NB * KT), dtype=np.float32)
        idx_cols = []
        gcol = 0
        for blocks in groups:
            for h, kh in ((0, K_LO), (1, K_HI)):
                flat = np.zeros(len(blocks) * kh * P, dtype=np.int64)
                for gi, b in enumerate(blocks):
                    e_src, e_dl, e_w = sub[(c, b, h)]
                    n = len(e_src)
                    o = gi * kh * P
                    flat[o : o + n] = e_src
                    # dw columns follow the same global column order
                    col0 = gcol + gi * kh
                    for t in range(kh):
                        lo_i = t * P
                        hi_i = min((t + 1) * P, n)
                        if lo_i < n:
                            cnt = hi_i - lo_i
                            dstloc[:cnt, col0 + t] = e_dl[lo_i:hi_i]
                            wedge[:cnt, col0 + t] = e_w[lo_i:hi_i]
                idx_cols.append(_wrap16(flat))
                gcol += len(blocks) * kh
        per_core.append(
            dict(
                idx16=np.ascontiguousarray(np.concatenate(idx_cols, axis=1)),
                dstloc=np.ascontiguousarray(dstloc),
                wedge=np.ascontiguousarray(wedge),
            )
        )
    return K_LO, K_HI, per_core


def _build_program(K_LO, K_HI):
    import concourse.bacc as bacc
    import concourse.mybir as mybir
    from concourse.masks import make_identity
    from concourse.tile import TileContext

    f32 = mybir.dt.float32
    i16 = mybir.dt.int16
    KT = K_LO + K_HI
    NBK = NB * KT
    groups = _groups()
    IDXC = NBK * P // 16  # idx16 total columns

    nc = bacc.Bacc("TRN2")
    featLoD = nc.dram_tensor("featlo", [HALF, D], f32, kind="ExternalInput")
    featHiD = nc.dram_tensor("feathi", [HALF, D], f32, kind="ExternalInput")
    wD = nc.dram_tensor("wmat", [P, 2 * D], f32, kind="ExternalInput")
    bD = nc.dram_tensor("bias2", [P, 2], f32, kind="ExternalInput")
    idxD = nc.dram_tensor("idx16", [P, IDXC], i16, kind="ExternalInput")
    dwD = nc.dram_tensor("dw", [P, 2 * NBK], f32, kind="ExternalInput")
    maskD = nc.dram_tensor("maskt", [D, NPAD], f32, kind="ExternalInput")
    outD = nc.dram_tensor("out", [NPAD, D], f32, kind="ExternalOutput")

    # stage-2 d-chunks
    chunks = []
    d0 = 0
    while d0 < NPAD:
        dn = min(512, NPAD - d0)
        chunks.append((d0, dn))
        d0 += dn

    with TileContext(nc) as tc:
        with (
            tc.tile_pool(name="const", bufs=1) as const,
            tc.tile_pool(name="gpool", bufs=2) as gpool,
            tc.tile_pool(name="ppool", bufs=4) as ppool,
            tc.tile_pool(name="psagg", bufs=2, space="PSUM") as psagg,
            tc.tile_pool(name="ps2", bufs=2, space="PSUM") as ps2p,
            tc.tile_pool(name="pst", bufs=2, space="PSUM") as pstp,
            tc.tile_pool(name="ypool", bufs=4) as ypool,
            tc.tile_pool(name="mpool", bufs=3) as mpool,
            tc.tile_pool(name="opool", bufs=3) as opool,
        ):
            # ---- constants / persistent tiles ----
            w_sb = const.tile([P, 2 * D], f32)  # [fi_in_half, fi_half*256+fo]
            nc.sync.dma_start(out=w_sb[:], in_=wD[:])
            b_sb = const.tile([P, 2], f32)
            nc.sync.dma_start(out=b_sb[:], in_=bD[:])
            iota_sb = const.tile([P, P], f32)
            nc.gpsimd.iota(
                iota_sb[:],
                pattern=[[1, P]],
                base=0,
                channel_multiplier=0,
                allow_small_or_imprecise_dtypes=True,
            )
            ident = const.tile([P, P], f32)
            make_identity(nc, ident[:])
            idx_sb = const.tile([P, IDXC], i16)
            nc.sync.dma_start(out=idx_sb[:], in_=idxD[:])
            dw_sb = const.tile([P, 2 * NBK], f32)  # [dstloc | wedge]
            nc.sync.dma_start(out=dw_sb[:], in_=dwD[:])
            # transposed aggregate: [fi(128) x (fi_half, dst_local 6272)]
            aggT = const.tile([P, 2 * NPAD], f32)

            # ---- stage 1: gather + weighted one-hot + matmul-accumulate ----
            gcol = 0  # global tile column
            icol = 0  # idx16 column offset (int16 cols)
            for blocks in groups:
                ng = len(blocks)
                ncols = ng * KT
                g = gpool.tile([P, ncols * D], f32, tag="g")
                # two gather calls: lo half then hi half
                for h, kh, tbl in ((0, K_LO, featLoD), (1, K_HI, featHiD)):
                    ni = ng * kh * P
                    cpos = 0 if h == 0 else ng * K_LO
                    nc.gpsimd.dma_gather(
                        out_ap=g[:, cpos * D : (cpos + ng * kh) * D].rearrange(
                            "p (k d) -> p k d", k=ng * kh
                        ),
                        in_ap=tbl[:],
                        idxs_ap=idx_sb[:, icol : icol + ni // 16],
                        num_idxs=ni,
                        num_idxs_reg=ni,
                        elem_size=D,
                        single_packet=False,
                    )
                    icol += ni // 16
                for gi, b in enumerate(blocks):
                    pt0 = psagg.tile([P, P], f32, tag="pt0")
                    pt1 = psagg.tile([P, P], f32, tag="pt1")
                    tiles = [  # (G column, dw column) for this block
                        (gi * K_LO + t, gcol + gi * K_LO + t)
                        for t in range(K_LO)
                    ] + [
                        (ng * K_LO + gi * K_HI + t, gcol + ng * K_LO + gi * K_HI + t)
                        for t in range(K_HI)
                    ]
                    for ti, (gc_, dc) in enumerate(tiles):
                        # ph[e, s] = (iota[e,s] == dstloc[e]) * wedge[e]
                        # (two ops: TensorScalarPtr's per-partition pointer
                        #  path is ~12x slower than line rate on HW)
                        pe_ = ppool.tile([P, P], f32, tag="pe_")
                        nc.vector.tensor_tensor(
                            out=pe_[:],
                            in0=iota_sb[:],
                            in1=dw_sb[:, dc : dc + 1].to_broadcast([P, P]),
                            op=mybir.AluOpType.is_equal,
                        )
                        ph = ppool.tile([P, P], f32, tag="ph")
                        nc.scalar.activation(
                            ph[:],
                            pe_[:],
                            mybir.ActivationFunctionType.Copy,
                            scale=dw_sb[:, NBK + dc : NBK + dc + 1],
                        )
                        nc.tensor.matmul(
                            pt0[:],
                            lhsT=g[:, gc_ * D : gc_ * D + P],
                            rhs=ph[:],
                            start=(ti == 0),
                            stop=(ti == KT - 1),
                        )
                        nc.tensor.matmul(
                            pt1[:],
                            lhsT=g[:, gc_ * D + P : (gc_ + 1) * D],
                            rhs=ph[:],
                            start=(ti == 0),
                            stop=(ti == KT - 1),
                        )
                    nc.vector.tensor_copy(aggT[:, b * P : (b + 1) * P], pt0[:])
                    nc.vector.tensor_copy(
                        aggT[:, NPAD + b * P : NPAD + (b + 1) * P], pt1[:]
                    )
                gcol += ncols

            # ---- stage 2: project, relu+bias, mask, transpose, store ----
            for d0, dn in chunks:
                yts = []
                for h in (0, 1):  # fo half
                    ps2 = ps2p.tile([P, 512], f32, tag="ps2")
                    nc.tensor.matmul(
                        ps2[:, :dn],
                        lhsT=w_sb[:, h * P : (h + 1) * P],
                        rhs=aggT[:, d0 : d0 + dn],
                        start=True,
                        stop=False,
                    )
                    nc.tensor.matmul(
                        ps2[:, :dn],
                        lhsT=w_sb[:, D + h * P : D + (h + 1) * P],
                        rhs=aggT[:, NPAD + d0 : NPAD + d0 + dn],
                        start=False,
                        stop=True,
                    )
                    yt = ypool.tile([P, 512], f32, tag="yt")
                    nc.scalar.activation(
                        yt[:, :dn],
                        ps2[:, :dn],
                        mybir.ActivationFunctionType.Relu,
                        bias=b_sb[:, h : h + 1],
                    )
                    mk = mpool.tile([P, 512], f32, tag="mk")
                    nc.sync.dma_start(
                        out=mk[:, :dn], in_=maskD[h * P : (h + 1) * P, d0 : d0 + dn]
                    )
                    nc.vector.tensor_mul(yt[:, :dn], yt[:, :dn], mk[:, :dn])
                    yts.append(yt)
                for jj in range(dn // P):
                    ot = opool.tile([P, D], f32, tag="ot")
                    for h in (0, 1):
                        ptr = pstp.tile([P, P], f32, tag="ptr")
                        nc.tensor.transpose(
                            ptr[:], yts[h][:, jj * P : (jj + 1) * P], ident[:]
                        )
                        nc.vector.tensor_copy(ot[:, h * P : (h + 1) * P], ptr[:])
                    nc.sync.dma_start(
                        out=outD[d0 + jj * P : d0 + (jj + 1) * P, :], in_=ot[:]
                    )
    nc.finalize()  # Bacc: runs compile() (register alloc, wait splitting)
    return nc


def kernel(feat, W, b, src, dst):
    feat = np.ascontiguousarray(np.asarray(feat, dtype=np.float32))
    W = np.asarray(W, dtype=np.float32)
    b = np.asarray(b, dtype=np.float32)

    K_LO, K_HI, per_core = _host_prep(src, dst)
    maskmul = _dropout_multiplier()
    maskT_all = np.ascontiguousarray(maskmul.T)  # [256, 50000]

    # W[fi, fo] -> [fi_in_half(128), fi_half*256 + fo]
    wmat = np.ascontiguousarray(
        W.reshape(2, P, D).transpose(1, 0, 2).reshape(P, 2 * D)
    )
    bias2 = np.ascontiguousarray(np.stack([b[:P], b[P:]], axis=1))
    featlo = np.ascontiguousarray(feat[:HALF])
    feathi = np.ascontiguousarray(feat[HALF:])

    in_maps = []
    for c in range(M):
        maskt = np.zeros((D, NPAD), dtype=np.float32)
        maskt[:, :NC_NODES] = maskT_all[:, c * NC_NODES : (c + 1) * NC_NODES]
        dw = np.concatenate(
            [per_core[c]["dstloc"], per_core[c]["wedge"]], axis=1
        )
        in_maps.append(
            dict(
                featlo=featlo,
                feathi=feathi,
                wmat=wmat,
                bias2=bias2,
                idx16=per_core[c]["idx16"],
                dw=np.ascontiguousarray(dw),
                maskt=maskt,
            )
        )

    nc = _build_program(K_LO, K_HI)

    from concourse.bass_utils import run_bass_kernel_spmd

    trace = bool(int(os.environ.get("KERNEL_TRACE", "0")))
    tmpdir = os.environ.get("KERNEL_TMPDIR") or None
    res = run_bass_kernel_spmd(
        nc, in_maps, core_ids=list(range(M)), trace=trace, tmpdir=tmpdir
    )
    if trace:
        kernel.last_exec_time_ns = res.exec_time_ns
        kernel.last_profile = res.profile_json

    out = np.empty((N_NODES, D), dtype=np.float32)
    for c in range(M):
        out[c * NC_NODES : (c + 1) * NC_NODES] = res.results[c]["out"][:NC_NODES]
    return out


# revision 21
# speedup vs baseline: 1.3299x; 1.0941x over previous
"""GCN layer (DGL GraphConv norm='both' + relu + dropout) on 8 trn2 NeuronCores.

Strategy: partition the 50000 nodes into 8 contiguous ranges of 6250 (one per
core). Every edge is owned by the core owning its dst node. Host-side prep is
index manipulation only (sort edges by dst, build padded per-block tile
arrays, degree bincounts -> per-edge norm weights); all feature math happens
on device:

  stage 1: batched dma_gather of feat[src] rows (feat is split into two
           <32768-row halves because gather indices are int16), weighted
           one-hot built on DVE in a single dual-scalar tensor_scalar op,
           PE matmuls accumulate the *transposed* aggregate
           aggT[fi, dst_local] in PSUM.
  stage 2: project with W (PE), relu(y+b) on ACT, dropout-mask multiply on
           DVE, PE transpose back to [node, feat], DMA out.

The dropout mask is a compile-time constant (jax threefry key 42, shape-only).
"""

import math
import os
import sys

import numpy as np

for _p in ("/opt/trn_rl_repo",):
    if _p not in sys.path and os.path.isdir(_p):
        sys.path.insert(0, _p)

N_NODES = 50000
N_EDGES = 400000
D = 256
DROPOUT_P = 0.1
M = 8  # cores
NC_NODES = N_NODES // M  # 6250
P = 128
NB = math.ceil(NC_NODES / P)  # 49 blocks (48 full + 106)
NPAD = NB * P  # 6272 padded nodes per core
HALF = N_NODES // 2  # feat table split for int16 gather indices
GRP = 4  # blocks per gather-call group

_mask_cache = {}


def _dropout_multiplier():
    """Exact replica of the reference's keep mask -> multiplier, f32."""
    if "m" not in _mask_cache:
        import jax

        cpu = jax.devices("cpu")[0]
        with jax.default_device(cpu):
            keep = np.asarray(
                jax.random.bernoulli(
                    jax.random.key(42), 1.0 - DROPOUT_P, (N_NODES, D)
                )
            )
        mult = np.where(
            keep, np.float32(1.0) / np.float32(1.0 - DROPOUT_P), np.float32(0.0)
        ).astype(np.float32)
        _mask_cache["m"] = mult
    return _mask_cache["m"]


def _groups():
    """Group block ids into gather groups of GRP blocks."""
    out = []
    b = 0
    while b < NB:
        out.append(list(range(b, min(b + GRP, NB))))
        b += GRP
    return out


def _wrap16(flat):
    """int16 gather index layout: position n -> [n%16, n//16], tiled to 128
    partitions (Q7 reads its copy per 16-partition group)."""
    C = len(flat) // 16
    a = np.zeros((16, C), dtype=np.int16)
    for n in range(len(flat)):
        a[n % 16, n // 16] = flat[n]
    return np.tile(a, (8, 1))


def _host_prep(src, dst):
    """Sort/shard edges, build per-core gather-index + one-hot arrays.

    Global tile-column order (matches G buffer columns and dw columns):
      for each group: [blk0 lo-tiles.. blkG-1 lo-tiles, blk0 hi-tiles, ...]
    Returns (K_LO, K_HI, per_core list of dicts).
    """
    src = np.asarray(src).astype(np.int64)
    dst = np.asarray(dst).astype(np.int64)

    deg_out = np.bincount(src, minlength=N_NODES).astype(np.float64)
    deg_in = np.bincount(dst, minlength=N_NODES).astype(np.float64)
    w_out = 1.0 / np.sqrt(np.maximum(deg_out, 1.0))
    w_in = 1.0 / np.sqrt(np.maximum(deg_in, 1.0))
    w_edge_all = (w_out[src] * w_in[dst]).astype(np.float32)

    order = np.argsort(dst, kind="stable")
    dst_s = dst[order]
    src_s = src[order]
    w_s = w_edge_all[order]

    # per (core, block, half): edge sublists
    sub = {}
    kmax = [1, 1]
    for c in range(M):
        base = c * NC_NODES
        for b in range(NB):
            lo = base + b * P
            hi = min(base + (b + 1) * P, (c + 1) * NC_NODES)
            s = np.searchsorted(dst_s, lo, side="left")
            e = np.searchsorted(dst_s, hi, side="left")
            sl_src = src_s[s:e]
            sl_dst = dst_s[s:e]
            sl_w = w_s[s:e]
            is_lo = sl_src < HALF
            for h in (0, 1):
                m_ = is_lo if h == 0 else ~is_lo
                sub[(c, b, h)] = (
                    sl_src[m_] - h * HALF,
                    (sl_dst[m_] - lo).astype(np.float32),
                    sl_w[m_],
                )
                kmax[h] = max(kmax[h], math.ceil(m_.sum() / P))
    K_LO, K_HI = kmax
    KT = K_LO + K_HI
    groups = _groups()

    per_core = []
    for c in range(M):
        dstloc = np.full((P, NB * KT), -1.0, dtype=np.float32)
        wedge = np.zeros((P, NB * KT), dtype=np.float32)
        idx_cols = []
        gcol = 0
        for blocks in groups:
            for h, kh in ((0, K_LO), (1, K_HI)):
                # sentinel -> row 0 (real read; P=0 nulls the contribution)
                flat = np.zeros(len(blocks) * kh * P, dtype=np.int64)
                for gi, b in enumerate(blocks):
                    e_src, e_dl, e_w = sub[(c, b, h)]
                    n = len(e_src)
                    o = gi * kh * P
                    flat[o : o + n] = e_src
                    # dw columns follow the same global column order
                    col0 = gcol + gi * kh
                    for t in range(kh):
                        lo_i = t * P
                        hi_i = min((t + 1) * P, n)
                        if lo_i < n:
                            cnt = hi_i - lo_i
                            dstloc[:cnt, col0 + t] = e_dl[lo_i:hi_i]
                            wedge[:cnt, col0 + t] = e_w[lo_i:hi_i]
                idx_cols.append(_wrap16(flat))
                gcol += len(blocks) * kh
        per_core.append(
            dict(
                idx16=np.ascontiguousarray(np.concatenate(idx_cols, axis=1)),
                dstloc=np.ascontiguousarray(dstloc),
                wedge=np.ascontiguousarray(wedge),
            )
        )
    return K_LO, K_HI, per_core


def _build_program(K_LO, K_HI):
    import concourse.bacc as bacc
    import concourse.mybir as mybir
    from concourse.masks import make_identity
    from concourse.tile import TileContext

    f32 = mybir.dt.float32
    i16 = mybir.dt.int16
    KT = K_LO + K_HI
    NBK = NB * KT
    groups = _groups()
    IDXC = NBK * P // 16  # idx16 total columns

    nc = bacc.Bacc("TRN2")
    featLoD = nc.dram_tensor("featlo", [HALF, D], f32, kind="ExternalInput")
    featHiD = nc.dram_tensor("feathi", [HALF, D], f32, kind="ExternalInput")
    wD = nc.dram_tensor("wmat", [P, 2 * D], f32, kind="ExternalInput")
    bD = nc.dram_tensor("bias2", [P, 2], f32, kind="ExternalInput")
    idxD = nc.dram_tensor("idx16", [P, IDXC], i16, kind="ExternalInput")
    dwD = nc.dram_tensor("dw", [P, 2 * NBK], f32, kind="ExternalInput")
    maskD = nc.dram_tensor("maskt", [D, NPAD], f32, kind="ExternalInput")
    outD = nc.dram_tensor("out", [NPAD, D], f32, kind="ExternalOutput")

    # stage-2 d-chunks
    chunks = []
    d0 = 0
    while d0 < NPAD:
        dn = min(512, NPAD - d0)
        chunks.append((d0, dn))
        d0 += dn

    with TileContext(nc) as tc:
        with (
            tc.tile_pool(name="const", bufs=1) as const,
            tc.tile_pool(name="gpool", bufs=2) as gpool,
            tc.tile_pool(name="ppool", bufs=4) as ppool,
            tc.tile_pool(name="psagg", bufs=2, space="PSUM") as psagg,
            tc.tile_pool(name="ps2", bufs=2, space="PSUM") as ps2p,
            tc.tile_pool(name="pst", bufs=2, space="PSUM") as pstp,
            tc.tile_pool(name="ypool", bufs=4) as ypool,
            tc.tile_pool(name="mpool", bufs=3) as mpool,
            tc.tile_pool(name="opool", bufs=3) as opool,
        ):
            # ---- constants / persistent tiles ----
            w_sb = const.tile([P, 2 * D], f32)  # [fi_in_half, fi_half*256+fo]
            nc.sync.dma_start(out=w_sb[:], in_=wD[:])
            b_sb = const.tile([P, 2], f32)
            nc.sync.dma_start(out=b_sb[:], in_=bD[:])
            iota_sb = const.tile([P, P], f32)
            nc.gpsimd.iota(
                iota_sb[:],
                pattern=[[1, P]],
                base=0,
                channel_multiplier=0,
                allow_small_or_imprecise_dtypes=True,
            )
            ident = const.tile([P, P], f32)
            make_identity(nc, ident[:])
            idx_sb = const.tile([P, IDXC], i16)
            nc.sync.dma_start(out=idx_sb[:], in_=idxD[:])
            dw_sb = const.tile([P, 2 * NBK], f32)  # [dstloc | wedge]
            nc.sync.dma_start(out=dw_sb[:], in_=dwD[:])

            # G buffers see skipped (sentinel) slots: zero both pool slots
            # once so stale data is always finite (P=0 nulls contributions).
            for _ in range(2):
                gz = gpool.tile([P, GRP * KT * D], f32, tag="g")
                nc.vector.memset(gz[:], 0.0)

            # ---- stage 1+2 pipelined per group of GRP blocks ----
            gcol = 0  # global tile column
            icol = 0  # idx16 column offset (int16 cols)
            for gi, blocks in enumerate(groups):
                ng = len(blocks)
                ncols = ng * KT
                g = gpool.tile([P, ncols * D], f32, tag="g")
                # two gather calls: lo half then hi half
                for h, kh, tbl in ((0, K_LO, featLoD), (1, K_HI, featHiD)):
                    ni = ng * kh * P
                    cpos = 0 if h == 0 else ng * K_LO
                    nc.gpsimd.dma_gather(
                        out_ap=g[:, cpos * D : (cpos + ng * kh) * D].rearrange(
                            "p (k d) -> p k d", k=ng * kh
                        ),
                        in_ap=tbl[:],
                        idxs_ap=idx_sb[:, icol : icol + ni // 16],
                        num_idxs=ni,
                        num_idxs_reg=ni,
                        elem_size=D,
                        single_packet=False,
                    )
                    icol += ni // 16
                aggc = ypool.tile([P, 2 * 512], f32, tag="aggc")
                for bi, b in enumerate(blocks):
                    pt0 = psagg.tile([P, P], f32, tag="pt0")
                    pt1 = psagg.tile([P, P], f32, tag="pt1")
                    tiles = [  # (G column, dw column) for this block
                        (bi * K_LO + t, gcol + bi * K_LO + t)
                        for t in range(K_LO)
                    ] + [
                        (ng * K_LO + bi * K_HI + t, gcol + ng * K_LO + bi * K_HI + t)
                        for t in range(K_HI)
                    ]
                    for ti, (gc_, dc) in enumerate(tiles):
                        # ph[e, s] = (iota[e,s] == dstloc[e]) * wedge[e]
                        # (two ops: TensorScalarPtr's per-partition pointer
                        #  path is ~12x slower than line rate on HW)
                        pe_ = ppool.tile([P, P], f32, tag="pe_")
                        nc.vector.tensor_tensor(
                            out=pe_[:],
                            in0=iota_sb[:],
                            in1=dw_sb[:, dc : dc + 1].to_broadcast([P, P]),
                            op=mybir.AluOpType.is_equal,
                        )
                        ph = ppool.tile([P, P], f32, tag="ph")
                        nc.scalar.activation(
                            ph[:],
                            pe_[:],
                            mybir.ActivationFunctionType.Copy,
                            scale=dw_sb[:, NBK + dc : NBK + dc + 1],
                        )
                        nc.tensor.matmul(
                            pt0[:],
                            lhsT=g[:, gc_ * D : gc_ * D + P],
                            rhs=ph[:],
                            start=(ti == 0),
                            stop=(ti == KT - 1),
                        )
                        nc.tensor.matmul(
                            pt1[:],
                            lhsT=g[:, gc_ * D + P : (gc_ + 1) * D],
                            rhs=ph[:],
                            start=(ti == 0),
                            stop=(ti == KT - 1),
                        )
                    nc.vector.tensor_copy(aggc[:, bi * P : (bi + 1) * P], pt0[:])
                    nc.vector.tensor_copy(
                        aggc[:, 512 + bi * P : 512 + (bi + 1) * P], pt1[:]
                    )
                gcol += ncols

                # ---- stage 2 for this group's dst chunk ----
                d0 = gi * GRP * P
                dn = ng * P
                yts = []
                for h in (0, 1):  # fo half
                    ps2 = ps2p.tile([P, 512], f32, tag="ps2")
                    nc.tensor.matmul(
                        ps2[:, :dn],
                        lhsT=w_sb[:, h * P : (h + 1) * P],
                        rhs=aggc[:, 0:dn],
                        start=True,
                        stop=False,
                    )
                    nc.tensor.matmul(
                        ps2[:, :dn],
                        lhsT=w_sb[:, D + h * P : D + (h + 1) * P],
                        rhs=aggc[:, 512 : 512 + dn],
                        start=False,
                        stop=True,
                    )
                    yt = ypool.tile([P, 512], f32, tag="yt")
                    nc.scalar.activation(
                        yt[:, :dn],
                        ps2[:, :dn],
                        mybir.ActivationFunctionType.Relu,
                        bias=b_sb[:, h : h + 1],
                    )
                    mk = mpool.tile([P, 512], f32, tag="mk")
                    nc.sync.dma_start(
                        out=mk[:, :dn], in_=maskD[h * P : (h + 1) * P, d0 : d0 + dn]
                    )
                    nc.vector.tensor_mul(yt[:, :dn], yt[:, :dn], mk[:, :dn])
                    yts.append(yt)
                for jj in range(dn // P):
                    ot = opool.tile([P, D], f32, tag="ot")
                    for h in (0, 1):
                        ptr = pstp.tile([P, P], f32, tag="ptr")
                        nc.tensor.transpose(
                            ptr[:], yts[h][:, jj * P : (jj + 1) * P], ident[:]
                        )
                        nc.vector.tensor_copy(ot[:, h * P : (h + 1) * P], ptr[:])
                    nc.sync.dma_start(
                        out=outD[d0 + jj * P : d0 + (jj + 1) * P, :], in_=ot[:]
                    )
    nc.finalize()  # Bacc: runs compile() (register alloc, wait splitting)
    return nc


def kernel(feat, W, b, src, dst):
    feat = np.ascontiguousarray(np.asarray(feat, dtype=np.float32))
    W = np.asarray(W, dtype=np.float32)
    b = np.asarray(b, dtype=np.float32)

    K_LO, K_HI, per_core = _host_prep(src, dst)
    maskmul = _dropout_multiplier()
    maskT_all = np.ascontiguousarray(maskmul.T)  # [256, 50000]

    # W[fi, fo] -> [fi_in_half(128), fi_half*256 + fo]
    wmat = np.ascontiguousarray(
        W.reshape(2, P, D).transpose(1, 0, 2).reshape(P, 2 * D)
    )
    bias2 = np.ascontiguousarray(np.stack([b[:P], b[P:]], axis=1))
    featlo = np.ascontiguousarray(feat[:HALF])
    feathi = np.ascontiguousarray(feat[HALF:])

    in_maps = []
    for c in range(M):
        maskt = np.zeros((D, NPAD), dtype=np.float32)
        maskt[:, :NC_NODES] = maskT_all[:, c * NC_NODES : (c + 1) * NC_NODES]
        dw = np.concatenate(
            [per_core[c]["dstloc"], per_core[c]["wedge"]], axis=1
        )
        in_maps.append(
            dict(
                featlo=featlo,
                feathi=feathi,
                wmat=wmat,
                bias2=bias2,
                idx16=per_core[c]["idx16"],
                dw=np.ascontiguousarray(dw),
                maskt=maskt,
            )
        )

    nc = _build_program(K_LO, K_HI)

    from concourse.bass_utils import run_bass_kernel_spmd

    trace = bool(int(os.environ.get("KERNEL_TRACE", "0")))
    tmpdir = os.environ.get("KERNEL_TMPDIR") or None
    res = run_bass_kernel_spmd(
        nc, in_maps, core_ids=list(range(M)), trace=trace, tmpdir=tmpdir
    )
    if trace:
        kernel.last_exec_time_ns = res.exec_time_ns
        kernel.last_profile = res.profile_json

    out = np.empty((N_NODES, D), dtype=np.float32)
    for c in range(M):
        out[c * NC_NODES : (c + 1) * NC_NODES] = res.results[c]["out"][:NC_NODES]
    return out
